# revision 10
# baseline (speedup 1.0000x reference)
"""Trainium2 Bass kernel for PersonalizedCalibrationNetwork (MoE-style judge routing).

Strategy: expert-parallel over the judge axis. Judge j lives on core j // 8.
The host routes samples to the core owning their judge, groups them by judge,
and pads every judge group to a uniform capacity C (so the single SPMD program
is shape-identical on all 8 cores). Each core computes, for its samples:

    z1 = sigmoid(x_aug @ (W1 + W1a[j]))      x_aug = [x, 1]
    z2 = sigmoid(z1_aug @ (W2 + W2a[j]))
    out = z2_aug @ (V + Va[j])               flattened to [257, 64]

All matmuls run transposed (features on partitions): z^T = G^T @ x^T, so layer
outputs feed the next layer without transposes. Judges are processed in
independent streams (groups of up to 4 sharing a PSUM bank strip); per group:
  - the shared weight part is a full-width matmul,
  - each judge's weight part accumulates into its column slice,
  - all bias rows (8 judge + 1 shared) are applied by ONE K=9 matmul against a
    host-built 0/1 block mask [9, N].
Streams are emitted depth-first (stream 0 runs layers 1-3 while stream 1's
weights arrive). Because each dma_start carries ~2us of ring-serial completion
latency, the host packs everything into 3 giant contiguous transfers: a "head"
blob (x^T | shared weights | biases+mask) and one all-layer weight blob per
stream. Inputs are bf16; accumulation is fp32 in PSUM.
"""

import ml_dtypes
import numpy as np

import concourse.mybir as mybir
import concourse.tile as tile
from concourse import bacc
from concourse.bass_utils import run_bass_kernel_spmd


class _SlimTileContext(tile.TileContext):
    """TileContext with a slimmer kernel tail: one all-engine barrier
    before the semaphore clears instead of two (each engine simply halts
    after the clears; NRT waits for all engines before NEFF completion)."""

    def _drain_and_barrier(self, tick_clock, wait_clock):
        drain_inst = self.nc.sync.drain()
        wait_clock.add_sem_waits(
            drain_inst.ins, tile.ScopedClock({None: tick_clock.global_clock}))
        self.nc.all_engine_barrier()
        popped = self.nc._tile_sem_poison_stack.pop()
        assert popped is self._sem_poison
        self.nc.clear_and_free_semaphores(
            list(self.sems.allocated().values()))


N_CORES = 8
J = 64                 # judges
JPC = J // N_CORES     # judges per core
IN = 256               # input features (+1 bias)
L1 = 256
L2 = 256
Q = 16
A = 4
QA = Q * A             # 64 output columns
P = 128                # partitions
PSUM_W = 512           # fp32 psum bank width
KB = JPC + 1           # bias-matmul contraction dim (8 judge rows + shared)
NB = L1 + L2 + QA      # bias pack columns
WJ = 2 * (L1 + L2 + QA)  # per-judge weight blob columns (1152)

BF16 = True
NP_W = ml_dtypes.bfloat16 if BF16 else np.float32

_cache = {}


def _make_groups(C):
    """Split the 8 judges into groups whose column strip fits a PSUM bank."""
    per_group = max(1, min(4, PSUM_W // C)) if C <= PSUM_W else 1
    groups = []  # (col0, gw, [(judge, ncol, width), ...])
    if C <= PSUM_W:
        for g0 in range(0, JPC, per_group):
            js = list(range(g0, min(g0 + per_group, JPC)))
            blocks = [(jj, jj * C, C) for jj in js]
            groups.append((g0 * C, len(js) * C, blocks))
    else:
        for jj in range(JPC):
            for c0 in range(0, C, PSUM_W):
                w = min(PSUM_W, C - c0)
                groups.append((jj * C + c0, w, [(jj, jj * C + c0, w)]))
    return groups


def _build_program(C):
    """Build + compile the SPMD Bass program for per-judge capacity C."""
    N = JPC * C  # padded samples per core
    groups = _make_groups(C)
    n_streams = len(groups)
    NH = 2 * N + 2 * NB + NB + N  # head cols: xT | wsh | bm(padded rows)

    nc = bacc.Bacc("TRN2", target_bir_lowering=False, debug=False,
                   num_devices=N_CORES)
    dt = mybir.dt.bfloat16 if BF16 else mybir.dt.float32
    f32 = mybir.dt.float32

    # head blob [P, NH]: cols [0:2N) x^T (ko-major), [2N:2N+1152) shared
    # weights (ko-major), [2N+1152:NH) bias+mask on partitions 0..8 only
    head_d = nc.dram_tensor("head", [P, NH], dt, kind="ExternalInput")
    # per-stream judge weights, section-major: A = layer-1, B = layer-2 | out
    wa_d = [nc.dram_tensor(f"wa{s}", [P, len(groups[s][2]) * 2 * L1], dt,
                           kind="ExternalInput") for s in range(n_streams)]
    wb_d = [nc.dram_tensor(f"wb{s}", [P, len(groups[s][2]) * 2 * (L2 + QA)],
                           dt, kind="ExternalInput") for s in range(n_streams)]
    out_d = nc.dram_tensor("outT", [QA, N], f32, kind="ExternalOutput")

    X0 = 0            # x^T base col: ko*N + n
    W0 = 2 * N        # shared weights base col: ko*NB + m
    B0 = 2 * N + 2 * NB   # bias base col (partitions 0..8): + m
    M0 = B0 + NB      # mask base col (partitions 0..8): + n

    with _SlimTileContext(nc) as tc:
        with (
            tc.tile_pool(name="const", bufs=1) as const,
            tc.tile_pool(name="psum", bufs=5, space="PSUM") as psum,
        ):
            head = const.tile([P, NH], dt, tag="head")
            wa = [const.tile([P, len(groups[s][2]) * 2 * L1], dt,
                             tag=f"wa{s}", name=f"wa{s}")
                  for s in range(n_streams)]
            wb = [const.tile([P, len(groups[s][2]) * 2 * (L2 + QA)], dt,
                             tag=f"wb{s}", name=f"wb{s}")
                  for s in range(n_streams)]
            z1T = const.tile([P, 2, N], dt, tag="z1T")
            z2T = const.tile([P, 2, N], dt, tag="z2T")
            outT = const.tile([QA, N], f32, tag="outT")

            # one serial chain on the sync ring, in exact need order:
            # concurrent DMAs round-robin the queues and delay the
            # first-needed tensor, so serialize instead
            nc.sync.dma_start(head[:], head_d[:])
            for s in range(n_streams):
                nc.sync.dma_start(wa[s][:], wa_d[s][:])
                nc.sync.dma_start(wb[s][:], wb_d[s][:])

            def glayer(s, li, rhs_of, M, zout):
                """One layer of stream s: z^T = act(W^T @ rhs + b).

                rhs_of(ko, c0, w) -> [128, w] rhs slice; li = layer index.
                """
                col0, gw, blocks = groups[s]
                sh_off = [0, L1, L1 + L2][li]
                wj_off = [0, 2 * L1, 2 * (L1 + L2)][li]
                n_m = (M + P - 1) // P
                for m in range(n_m):
                    mw = min(P, M - m * P)
                    ps = psum.tile([P, PSUM_W], f32, tag="ps",
                                   name="ps")[:mw, :gw]
                    for ko in range(2):
                        nc.tensor.matmul(
                            ps,
                            head[:, W0 + ko * NB + sh_off + m * P:
                                 W0 + ko * NB + sh_off + m * P + mw],
                            rhs_of(ko, col0, gw),
                            start=(ko == 0), stop=False)
                    nc.tensor.matmul(
                        ps,
                        head[:KB, B0 + sh_off + m * P:B0 + sh_off + m * P + mw],
                        head[:KB, M0 + col0:M0 + col0 + gw],
                        start=False, stop=False)
                    wt = wa[s] if li == 0 else wb[s]
                    msz = mwof(li)
                    sec = 0 if li < 2 else len(blocks) * 2 * L2
                    for bi, (jj, ncol, w) in enumerate(blocks):
                        off = ncol - col0
                        base = sec + bi * 2 * msz
                        for ko in range(2):
                            nc.tensor.matmul(
                                ps[:, off:off + w],
                                wt[:, base + ko * msz + m * P:
                                   base + ko * msz + m * P + mw],
                                rhs_of(ko, ncol, w),
                                start=False,
                                stop=(bi == len(blocks) - 1 and ko == 1))
                    if zout is not None:
                        nc.scalar.activation(
                            zout[:mw, m, col0:col0 + gw], ps,
                            mybir.ActivationFunctionType.Sigmoid)
                    else:
                        nc.vector.tensor_copy(
                            outT[:mw, col0:col0 + gw], ps)

            def mwof(li):  # per-layer output width (cols per ko in blob)
                return [L1, L2, QA][li]

            def rhs_x(ko, c0, w):
                return head[:, X0 + ko * N + c0:X0 + ko * N + c0 + w]

            def rhs_of_tile(t):
                return lambda ko, c0, w: t[:, ko, c0:c0 + w]

            for s in range(n_streams):
                glayer(s, 0, rhs_x, L1, z1T)
                glayer(s, 1, rhs_of_tile(z1T), L2, z2T)
                glayer(s, 2, rhs_of_tile(z2T), QA, None)

            nc.scalar.dma_start(out_d[:], outT[:])

    nc.compile()
    return nc, N, groups


def kernel(X_machine_evals, X_human_judges, W1, W1a, W2, W2a, V, Va):
    X = np.asarray(X_machine_evals, dtype=np.float32)
    jid = np.asarray(X_human_judges).reshape(-1).astype(np.int64)
    W1 = np.asarray(W1, dtype=np.float32)
    W1a = np.asarray(W1a, dtype=np.float32)
    W2 = np.asarray(W2, dtype=np.float32)
    W2a = np.asarray(W2a, dtype=np.float32)
    V = np.asarray(V, dtype=np.float32)
    Va = np.asarray(Va, dtype=np.float32)
    B = X.shape[0]

    counts = np.bincount(jid, minlength=J)
    C = int(counts.max())

    if C not in _cache:
        _cache[C] = _build_program(C)
    nc, N, groups = _cache[C]

    # stable order of sample indices grouped by judge
    order = np.argsort(jid, kind="stable")
    sorted_j = jid[order]

    def pack_w(w):  # [256, M] -> [128, 2*M] (ko-major cols)
        M = w.shape[1]
        return w[:256].reshape(2, P, M).transpose(1, 0, 2).reshape(P, 2 * M)

    Vf = V.transpose(1, 0, 2).reshape(IN + 1, QA)          # [257, 64]
    Vaf = Va.transpose(0, 2, 1, 3).reshape(J, IN + 1, QA)  # [J, 257, 64]

    # shared-weight block [2, P, NB] -> [P, 2*NB] ko-major
    wsh_cols = np.concatenate(
        [W1[:256].reshape(2, P, L1), W2[:256].reshape(2, P, L2),
         Vf[:256].reshape(2, P, QA)], axis=2)
    wsh_flat = wsh_cols.transpose(1, 0, 2).reshape(P, 2 * NB)

    mask_in = np.zeros((KB, N), dtype=np.float32)
    mask_in[JPC, :] = 1
    for k in range(JPC):
        mask_in[k, k * C:(k + 1) * C] = 1

    NH = 2 * N + 2 * NB + NB + N

    in_maps = []
    core_meta = []
    for c in range(N_CORES):
        judges = np.arange(c * JPC, (c + 1) * JPC)
        Xp = np.zeros((N, IN), dtype=np.float32)
        samp = []  # per-judge sample indices
        for k, jj in enumerate(judges):
            idx = order[np.searchsorted(sorted_j, jj):
                        np.searchsorted(sorted_j, jj, side="right")]
            Xp[k * C:k * C + len(idx)] = X[idx]
            samp.append(idx)
        core_meta.append(samp)

        head = np.zeros((P, NH), dtype=np.float32)
        head[:, :2 * N] = Xp.T.reshape(2, P, N).transpose(1, 0, 2).reshape(
            P, 2 * N)
        head[:, 2 * N:2 * N + 2 * NB] = wsh_flat
        b0 = 2 * N + 2 * NB
        head[:JPC, b0:b0 + L1] = W1a[judges, 256]
        head[:JPC, b0 + L1:b0 + L1 + L2] = W2a[judges, 256]
        head[:JPC, b0 + L1 + L2:b0 + NB] = Vaf[judges, 256]
        head[JPC, b0:b0 + NB] = np.concatenate([W1[256], W2[256], Vf[256]])
        head[:KB, b0 + NB:] = mask_in

        im = {"head": head.astype(NP_W)}
        for s, (_, _, blocks) in enumerate(groups):
            js = judges[[b[0] for b in blocks]]
            wa_blob = np.concatenate([pack_w(W1a[jj]) for jj in js], axis=1)
            wb_blob = np.concatenate(
                [pack_w(W2a[jj]) for jj in js] +
                [pack_w(Vaf[jj]) for jj in js], axis=1)
            im[f"wa{s}"] = np.ascontiguousarray(wa_blob.astype(NP_W))
            im[f"wb{s}"] = np.ascontiguousarray(wb_blob.astype(NP_W))
        in_maps.append(im)

    res = run_bass_kernel_spmd(nc, in_maps, core_ids=list(range(N_CORES)))

    out = np.zeros((B, Q, A), dtype=np.float32)
    for c in range(N_CORES):
        oT = res.results[c]["outT"]          # [64, N]
        o = oT.T.reshape(N, Q, A)
        for k, idx in enumerate(core_meta[c]):
            out[idx] = o[k * C:k * C + len(idx)]
    return out


# revision 11
# speedup vs baseline: 1.1271x; 1.1271x over previous
"""Trainium2 Bass kernel for PersonalizedCalibrationNetwork (MoE-style judge routing).

Strategy: expert-parallel over the judge axis. Judge j lives on core j // 8.
The host routes samples to the core owning their judge, groups them by judge,
and pads every judge group to a uniform capacity C (so the single SPMD program
is shape-identical on all 8 cores). Each core computes, for its samples:

    z1 = sigmoid(x_aug @ (W1 + W1a[j]))      x_aug = [x, 1]
    z2 = sigmoid(z1_aug @ (W2 + W2a[j]))
    out = z2_aug @ (V + Va[j])               flattened to [257, 64]

All matmuls run transposed (features on partitions): z^T = G^T @ x^T, so layer
outputs feed the next layer without transposes. Judges are processed in
independent streams (groups of up to 4 sharing a PSUM bank strip); per group:
  - the shared weight part is a full-width matmul,
  - each judge's weight part accumulates into its column slice,
  - all bias rows (8 judge + 1 shared) are applied by ONE K=9 matmul against a
    host-built 0/1 block mask [9, N].
Layer-1 shared matmuls for every stream run first (they only need the head
blob); each stream then runs judges + layers 2-3 in DMA-arrival order. The DMA
queues round-robin all outstanding transfers, so loads form two parallel
chains (sync and scalar HWDGE rings) in per-chain need order, split so each
stream's layer-1 weights land before its layer-2|3 blob. Inputs are bf16
(fp32 accumulation in PSUM); every transfer is one contiguous blob.
"""

import ml_dtypes
import numpy as np

import concourse.mybir as mybir
import concourse.tile as tile
from concourse import bacc
from concourse.bass_utils import run_bass_kernel_spmd


class _SlimTileContext(tile.TileContext):
    """TileContext with a slimmer kernel tail: one all-engine barrier
    before the semaphore clears instead of two (each engine simply halts
    after the clears; NRT waits for all engines before NEFF completion)."""

    def _drain_and_barrier(self, tick_clock, wait_clock):
        drain_inst = self.nc.sync.drain()
        wait_clock.add_sem_waits(
            drain_inst.ins, tile.ScopedClock({None: tick_clock.global_clock}))
        self.nc.all_engine_barrier()
        popped = self.nc._tile_sem_poison_stack.pop()
        assert popped is self._sem_poison
        self.nc.clear_and_free_semaphores(
            list(self.sems.allocated().values()))


N_CORES = 8
J = 64                 # judges
JPC = J // N_CORES     # judges per core
IN = 256               # input features (+1 bias)
L1 = 256
L2 = 256
Q = 16
A = 4
QA = Q * A             # 64 output columns
P = 128                # partitions
PSUM_W = 512           # fp32 psum bank width
KB = JPC + 1           # bias-matmul contraction dim (8 judge rows + shared)
NB = L1 + L2 + QA      # bias pack columns

BF16 = True
NP_W = ml_dtypes.bfloat16 if BF16 else np.float32

_cache = {}


def _make_groups(C):
    """Split the 8 judges into groups whose column strip fits a PSUM bank."""
    per_group = max(1, min(4, PSUM_W // C)) if C <= PSUM_W else 1
    groups = []  # (col0, gw, [(judge, ncol, width), ...])
    if C <= PSUM_W:
        for g0 in range(0, JPC, per_group):
            js = list(range(g0, min(g0 + per_group, JPC)))
            blocks = [(jj, jj * C, C) for jj in js]
            groups.append((g0 * C, len(js) * C, blocks))
    else:
        for jj in range(JPC):
            for c0 in range(0, C, PSUM_W):
                w = min(PSUM_W, C - c0)
                groups.append((jj * C + c0, w, [(jj, jj * C + c0, w)]))
    return groups


def _stream_order(n):
    """Streams in expected DMA-arrival order (odd = scalar chain first)."""
    order = []
    for pair in range((n + 1) // 2):
        if 2 * pair + 1 < n:
            order.append(2 * pair + 1)
        order.append(2 * pair)
    return order


def _build_program(C):
    """Build + compile the SPMD Bass program for per-judge capacity C."""
    N = JPC * C  # padded samples per core
    groups = _make_groups(C)
    n_streams = len(groups)
    NH = 2 * NB + 2 * N  # head cols: shared weights | x^T

    nc = bacc.Bacc("TRN2", target_bir_lowering=False, debug=False,
                   num_devices=N_CORES)
    dt = mybir.dt.bfloat16 if BF16 else mybir.dt.float32
    f32 = mybir.dt.float32

    # head blob [P, NH]: cols [0:2NB) shared weights (ko-major),
    # [2NB:NH) x^T (ko-major)
    head_d = nc.dram_tensor("head", [P, NH], dt, kind="ExternalInput")
    # bias rows + block mask [KB, NB + N]
    bm_d = nc.dram_tensor("bm", [KB, NB + N], dt, kind="ExternalInput")
    # per-stream judge weights, section-major: A = layer-1, B = layer-2 | out
    wa_d = [nc.dram_tensor(f"wa{s}", [P, len(groups[s][2]) * 2 * L1], dt,
                           kind="ExternalInput") for s in range(n_streams)]
    wb_d = [nc.dram_tensor(f"wb{s}", [P, len(groups[s][2]) * 2 * (L2 + QA)],
                           dt, kind="ExternalInput") for s in range(n_streams)]
    out_d = nc.dram_tensor("outT", [QA, N], f32, kind="ExternalOutput")

    W0 = 0            # shared weights base col: ko*NB + m
    X0 = 2 * NB       # x^T base col: ko*N + n

    with _SlimTileContext(nc) as tc:
        with (
            tc.tile_pool(name="const", bufs=1) as const,
            tc.tile_pool(name="psum", bufs=6, space="PSUM") as psum,
        ):
            head = const.tile([P, NH], dt, tag="head")
            bm = const.tile([KB, NB + N], dt, tag="bm")
            wa = [const.tile([P, len(groups[s][2]) * 2 * L1], dt,
                             tag=f"wa{s}", name=f"wa{s}")
                  for s in range(n_streams)]
            wb = [const.tile([P, len(groups[s][2]) * 2 * (L2 + QA)], dt,
                             tag=f"wb{s}", name=f"wb{s}")
                  for s in range(n_streams)]
            z1T = const.tile([P, 2, N], dt, tag="z1T")
            z2T = const.tile([P, 2, N], dt, tag="z2T")
            outT = const.tile([QA, N], f32, tag="outT")

            # Two parallel chains in per-chain need order. Even streams ride
            # sync (behind head), odd streams ride scalar (behind bm).
            ring = [nc.sync, nc.scalar]
            nc.sync.dma_start(head[:], head_d[:])
            nc.scalar.dma_start(bm[:], bm_d[:])
            for pair in range((n_streams + 1) // 2):
                for s in (2 * pair, 2 * pair + 1):
                    if s < n_streams:
                        ring[s % 2].dma_start(wa[s][:], wa_d[s][:])
                for s in (2 * pair, 2 * pair + 1):
                    if s < n_streams:
                        ring[s % 2].dma_start(wb[s][:], wb_d[s][:])

            def rhs_x(ko, c0, w):
                return head[:, X0 + ko * N + c0:X0 + ko * N + c0 + w]

            def rhs_of_tile(t):
                return lambda ko, c0, w: t[:, ko, c0:c0 + w]

            def shared_part(s, li, rhs_of, M):
                """Open psum tiles for (stream, layer): shared + bias mms."""
                col0, gw, _ = groups[s]
                sh_off = [0, L1, L1 + L2][li]
                pss = []
                for m in range((M + P - 1) // P):
                    mw = min(P, M - m * P)
                    ps = psum.tile([P, PSUM_W], f32, tag="ps",
                                   name="ps")[:mw, :gw]
                    for ko in range(2):
                        nc.tensor.matmul(
                            ps,
                            head[:, W0 + ko * NB + sh_off + m * P:
                                 W0 + ko * NB + sh_off + m * P + mw],
                            rhs_of(ko, col0, gw),
                            start=(ko == 0), stop=False)
                    nc.tensor.matmul(
                        ps, bm[:, sh_off + m * P:sh_off + m * P + mw],
                        bm[:, NB + col0:NB + col0 + gw],
                        start=False, stop=False)
                    pss.append(ps)
                return pss

            def judge_part(s, li, rhs_of, M, zout, pss):
                """Judge mms into the open psum tiles, then act/copy."""
                col0, gw, blocks = groups[s]
                msz = [L1, L2, QA][li]
                wt = wa[s] if li == 0 else wb[s]
                sec = 0 if li < 2 else len(blocks) * 2 * L2
                for m, ps in enumerate(pss):
                    mw = min(P, M - m * P)
                    for bi, (jj, ncol, w) in enumerate(blocks):
                        off = ncol - col0
                        base = sec + bi * 2 * msz
                        for ko in range(2):
                            nc.tensor.matmul(
                                ps[:, off:off + w],
                                wt[:, base + ko * msz + m * P:
                                   base + ko * msz + m * P + mw],
                                rhs_of(ko, ncol, w),
                                start=False,
                                stop=(bi == len(blocks) - 1 and ko == 1))
                    if zout is not None:
                        nc.scalar.activation(
                            zout[:mw, m, col0:col0 + gw], ps,
                            mybir.ActivationFunctionType.Sigmoid)
                    else:
                        nc.vector.tensor_copy(
                            outT[:mw, col0:col0 + gw], ps)

            order = _stream_order(n_streams)
            # layer-1 shared matmuls for every stream first (head-only deps)
            l1ps = {s: shared_part(s, 0, rhs_x, L1) for s in order}
            for s in order:
                judge_part(s, 0, rhs_x, L1, z1T, l1ps[s])
                ps2 = shared_part(s, 1, rhs_of_tile(z1T), L2)
                judge_part(s, 1, rhs_of_tile(z1T), L2, z2T, ps2)
                ps3 = shared_part(s, 2, rhs_of_tile(z2T), QA)
                judge_part(s, 2, rhs_of_tile(z2T), QA, None, ps3)

            nc.scalar.dma_start(out_d[:], outT[:])

    nc.compile()
    return nc, N, groups


def kernel(X_machine_evals, X_human_judges, W1, W1a, W2, W2a, V, Va):
    X = np.asarray(X_machine_evals, dtype=np.float32)
    jid = np.asarray(X_human_judges).reshape(-1).astype(np.int64)
    W1 = np.asarray(W1, dtype=np.float32)
    W1a = np.asarray(W1a, dtype=np.float32)
    W2 = np.asarray(W2, dtype=np.float32)
    W2a = np.asarray(W2a, dtype=np.float32)
    V = np.asarray(V, dtype=np.float32)
    Va = np.asarray(Va, dtype=np.float32)
    B = X.shape[0]

    counts = np.bincount(jid, minlength=J)
    C = int(counts.max())

    if C not in _cache:
        _cache[C] = _build_program(C)
    nc, N, groups = _cache[C]

    # stable order of sample indices grouped by judge
    order = np.argsort(jid, kind="stable")
    sorted_j = jid[order]

    def pack_w(w):  # [256, M] -> [128, 2*M] (ko-major cols)
        M = w.shape[1]
        return w[:256].reshape(2, P, M).transpose(1, 0, 2).reshape(P, 2 * M)

    Vf = V.transpose(1, 0, 2).reshape(IN + 1, QA)          # [257, 64]
    Vaf = Va.transpose(0, 2, 1, 3).reshape(J, IN + 1, QA)  # [J, 257, 64]

    # shared-weight block [2, P, NB] -> [P, 2*NB] ko-major
    wsh_cols = np.concatenate(
        [W1[:256].reshape(2, P, L1), W2[:256].reshape(2, P, L2),
         Vf[:256].reshape(2, P, QA)], axis=2)
    wsh_flat = wsh_cols.transpose(1, 0, 2).reshape(P, 2 * NB)

    mask_in = np.zeros((KB, N), dtype=np.float32)
    mask_in[JPC, :] = 1
    for k in range(JPC):
        mask_in[k, k * C:(k + 1) * C] = 1

    in_maps = []
    core_meta = []
    for c in range(N_CORES):
        judges = np.arange(c * JPC, (c + 1) * JPC)
        Xp = np.zeros((N, IN), dtype=np.float32)
        samp = []  # per-judge sample indices
        for k, jj in enumerate(judges):
            idx = order[np.searchsorted(sorted_j, jj):
                        np.searchsorted(sorted_j, jj, side="right")]
            Xp[k * C:k * C + len(idx)] = X[idx]
            samp.append(idx)
        core_meta.append(samp)

        head = np.concatenate(
            [wsh_flat,
             Xp.T.reshape(2, P, N).transpose(1, 0, 2).reshape(P, 2 * N)],
            axis=1)
        bm_in = np.empty((KB, NB + N), dtype=np.float32)
        bm_in[:JPC, :L1] = W1a[judges, 256]
        bm_in[:JPC, L1:L1 + L2] = W2a[judges, 256]
        bm_in[:JPC, L1 + L2:NB] = Vaf[judges, 256]
        bm_in[JPC, :NB] = np.concatenate([W1[256], W2[256], Vf[256]])
        bm_in[:, NB:] = mask_in

        im = {"head": np.ascontiguousarray(head.astype(NP_W)),
              "bm": bm_in.astype(NP_W)}
        for s, (_, _, blocks) in enumerate(groups):
            js = judges[[b[0] for b in blocks]]
            wa_blob = np.concatenate([pack_w(W1a[jj]) for jj in js], axis=1)
            wb_blob = np.concatenate(
                [pack_w(W2a[jj]) for jj in js] +
                [pack_w(Vaf[jj]) for jj in js], axis=1)
            im[f"wa{s}"] = np.ascontiguousarray(wa_blob.astype(NP_W))
            im[f"wb{s}"] = np.ascontiguousarray(wb_blob.astype(NP_W))
        in_maps.append(im)

    res = run_bass_kernel_spmd(nc, in_maps, core_ids=list(range(N_CORES)))

    out = np.zeros((B, Q, A), dtype=np.float32)
    for c in range(N_CORES):
        oT = res.results[c]["outT"]          # [64, N]
        o = oT.T.reshape(N, Q, A)
        for k, idx in enumerate(core_meta[c]):
            out[idx] = o[k * C:k * C + len(idx)]
    return out


# revision 12
# speedup vs baseline: 1.1290x; 1.0017x over previous
"""Trainium2 Bass kernel for PersonalizedCalibrationNetwork (MoE-style judge routing).

Strategy: expert-parallel over the judge axis. Judge j lives on core j // 8.
The host routes samples to the core owning their judge, groups them by judge,
and pads every judge group to a uniform capacity C (so the single SPMD program
is shape-identical on all 8 cores). Each core computes, for its samples:

    z1 = sigmoid(x_aug @ (W1 + W1a[j]))      x_aug = [x, 1]
    z2 = sigmoid(z1_aug @ (W2 + W2a[j]))
    out = z2_aug @ (V + Va[j])               flattened to [257, 64]

All matmuls run transposed (features on partitions): z^T = G^T @ x^T, so layer
outputs feed the next layer without transposes. Per PSUM group (a bank-wide
strip of judge column-blocks):
  - the shared weight part is a full-width matmul,
  - each judge's weight part accumulates into its column slice,
  - all bias rows (8 judge + 1 shared) are applied by ONE K=9 matmul against a
    host-built 0/1 block mask [9, N].
Inputs are bf16 (fp32 accumulation in PSUM); host pre-packs every tensor in
the exact [128-partition, free] SBUF layout so every DMA is one contiguous
transfer, and the 6 input transfers are spread over 3 DGE rings (sync,
scalar, gpsimd) — each dma_start carries ~2us of completion latency, so few
big DMAs on parallel rings beat many small ones.
"""

import ml_dtypes
import numpy as np

import concourse.mybir as mybir
import concourse.tile as tile
from concourse import bacc
from concourse.bass_utils import run_bass_kernel_spmd


class _SlimTileContext(tile.TileContext):
    """TileContext with a slimmer kernel tail: one all-engine barrier
    before the semaphore clears instead of two (each engine simply halts
    after the clears; NRT waits for all engines before NEFF completion)."""

    def _drain_and_barrier(self, tick_clock, wait_clock):
        drain_inst = self.nc.sync.drain()
        wait_clock.add_sem_waits(
            drain_inst.ins, tile.ScopedClock({None: tick_clock.global_clock}))
        self.nc.all_engine_barrier()
        popped = self.nc._tile_sem_poison_stack.pop()
        assert popped is self._sem_poison
        self.nc.clear_and_free_semaphores(
            list(self.sems.allocated().values()))


N_CORES = 8
J = 64                 # judges
JPC = J // N_CORES     # judges per core
IN = 256               # input features (+1 bias)
L1 = 256
L2 = 256
Q = 16
A = 4
QA = Q * A             # 64 output columns
P = 128                # partitions
PSUM_W = 512           # fp32 psum bank width
KB = JPC + 1           # bias-matmul contraction dim (8 judge rows + shared)
NB = L1 + L2 + QA      # bias pack columns

BF16 = True
NP_W = ml_dtypes.bfloat16 if BF16 else np.float32

_cache = {}


def _make_groups(C):
    """Split the 8 judges into groups whose column strip fits a PSUM bank."""
    per_group = max(1, min(JPC, PSUM_W // C)) if C <= PSUM_W else 1
    groups = []  # (col0, gw, [(judge, ncol, width), ...])
    if C <= PSUM_W:
        for g0 in range(0, JPC, per_group):
            js = list(range(g0, min(g0 + per_group, JPC)))
            blocks = [(jj, jj * C, C) for jj in js]
            groups.append((g0 * C, len(js) * C, blocks))
    else:
        for jj in range(JPC):
            for c0 in range(0, C, PSUM_W):
                w = min(PSUM_W, C - c0)
                groups.append((jj * C + c0, w, [(jj, jj * C + c0, w)]))
    return groups


def _build_program(C):
    """Build + compile the SPMD Bass program for per-judge capacity C."""
    N = JPC * C  # padded samples per core
    groups = _make_groups(C)

    nc = bacc.Bacc("TRN2", target_bir_lowering=False, debug=False,
                   num_devices=N_CORES)
    dt = mybir.dt.bfloat16 if BF16 else mybir.dt.float32
    f32 = mybir.dt.float32

    # DRAM inputs, pre-packed host-side in SBUF layout
    xT_d = nc.dram_tensor("xT", [P, 2, N], dt, kind="ExternalInput")
    # shared weights [P, 2, 576]: cols [0:256) W1, [256:512) W2, [512:576) Vf
    wsh_d = nc.dram_tensor("wsh", [P, 2, NB], dt, kind="ExternalInput")
    w1a_d = nc.dram_tensor("w1a", [P, JPC, 2, L1], dt, kind="ExternalInput")
    w2a_d = nc.dram_tensor("w2a", [P, JPC, 2, L2], dt, kind="ExternalInput")
    va_d = nc.dram_tensor("va", [P, JPC, 2, QA], dt, kind="ExternalInput")
    # bias rows + block mask combined [KB, NB + N]:
    #   cols [0:NB) bias (rows 0..7 judge, row 8 shared),
    #   cols [NB:NB+N) mask (row jj = 1 on judge jj's columns, row 8 = ones)
    bm_d = nc.dram_tensor("bm", [KB, NB + N], dt, kind="ExternalInput")
    out_d = nc.dram_tensor("outT", [QA, N], f32, kind="ExternalOutput")

    with _SlimTileContext(nc) as tc:
        with (
            tc.tile_pool(name="const", bufs=1) as const,
            tc.tile_pool(name="psum", bufs=6, space="PSUM") as psum,
        ):
            xT = const.tile([P, 2, N], dt, tag="xT")
            wsh = const.tile([P, 2, NB], dt, tag="wsh")
            bm = const.tile([KB, NB + N], dt, tag="bm")
            w1a = const.tile([P, JPC, 2, L1], dt, tag="w1a")
            w2a = const.tile([P, JPC, 2, L2], dt, tag="w2a")
            va = const.tile([P, JPC, 2, QA], dt, tag="va")
            z1T = const.tile([P, 2, N], dt, tag="z1T")
            z2T = const.tile([P, 2, N], dt, tag="z2T")
            outT = const.tile([QA, N], f32, tag="outT")

            # spread the loads over 3 DGE rings, first-use order
            nc.sync.dma_start(xT[:], xT_d[:])
            nc.scalar.dma_start(wsh[:], wsh_d[:])
            nc.gpsimd.dma_start(bm[:], bm_d[:])
            nc.sync.dma_start(w1a[:], w1a_d[:])
            nc.scalar.dma_start(w2a[:], w2a_d[:])
            nc.gpsimd.dma_start(va[:], va_d[:])

            def layer(sh_off, w_jd, rhs, M, zout):
                """z^T[M, N] = act(W^T @ rhs + b), accumulated per group."""
                n_m = (M + P - 1) // P
                for col0, gw, blocks in groups:
                    for m in range(n_m):
                        mw = min(P, M - m * P)
                        ps = psum.tile([P, PSUM_W], f32, tag="ps",
                                       name="ps")[:mw, :gw]
                        ms = slice(sh_off + m * P, sh_off + m * P + mw)
                        for ko in range(2):
                            nc.tensor.matmul(
                                ps, wsh[:, ko, ms],
                                rhs[:, ko, col0:col0 + gw],
                                start=(ko == 0), stop=False)
                        nc.tensor.matmul(
                            ps, bm[:, sh_off + m * P:sh_off + m * P + mw],
                            bm[:, NB + col0:NB + col0 + gw],
                            start=False, stop=False)
                        for bi, (jj, ncol, w) in enumerate(blocks):
                            off = ncol - col0
                            for ko in range(2):
                                nc.tensor.matmul(
                                    ps[:, off:off + w],
                                    w_jd[:, jj, ko, m * P:m * P + mw],
                                    rhs[:, ko, ncol:ncol + w],
                                    start=False,
                                    stop=(bi == len(blocks) - 1 and ko == 1))
                        if zout is not None:
                            nc.scalar.activation(
                                zout[:mw, m, col0:col0 + gw], ps,
                                mybir.ActivationFunctionType.Sigmoid)
                        else:
                            nc.vector.tensor_copy(
                                outT[:mw, col0:col0 + gw], ps)

            layer(0, w1a, xT, L1, z1T)
            layer(L1, w2a, z1T, L2, z2T)
            layer(L1 + L2, va, z2T, QA, None)

            nc.sync.dma_start(out_d[:], outT[:])

    nc.compile()
    return nc, N, groups


def kernel(X_machine_evals, X_human_judges, W1, W1a, W2, W2a, V, Va):
    X = np.asarray(X_machine_evals, dtype=np.float32)
    jid = np.asarray(X_human_judges).reshape(-1).astype(np.int64)
    W1 = np.asarray(W1, dtype=np.float32)
    W1a = np.asarray(W1a, dtype=np.float32)
    W2 = np.asarray(W2, dtype=np.float32)
    W2a = np.asarray(W2a, dtype=np.float32)
    V = np.asarray(V, dtype=np.float32)
    Va = np.asarray(Va, dtype=np.float32)
    B = X.shape[0]

    counts = np.bincount(jid, minlength=J)
    C = int(counts.max())

    if C not in _cache:
        _cache[C] = _build_program(C)
    nc, N, groups = _cache[C]

    # stable order of sample indices grouped by judge
    order = np.argsort(jid, kind="stable")
    sorted_j = jid[order]

    def pack_w(w):  # [256, M] -> [128, 2, M]
        M = w.shape[1]
        return np.ascontiguousarray(
            w[:256].reshape(2, P, M).transpose(1, 0, 2).astype(NP_W))

    Vf = V.transpose(1, 0, 2).reshape(IN + 1, QA)          # [257, 64]
    Vaf = Va.transpose(0, 2, 1, 3).reshape(J, IN + 1, QA)  # [J, 257, 64]

    wsh_in = np.ascontiguousarray(
        np.concatenate([pack_w(W1), pack_w(W2), pack_w(Vf)], axis=2))

    mask_in = np.zeros((KB, N), dtype=np.float32)
    mask_in[JPC, :] = 1
    for k in range(JPC):
        mask_in[k, k * C:(k + 1) * C] = 1

    in_maps = []
    core_meta = []
    for c in range(N_CORES):
        judges = np.arange(c * JPC, (c + 1) * JPC)
        Xp = np.zeros((N, IN), dtype=np.float32)
        samp = []  # per-judge sample indices
        for k, jj in enumerate(judges):
            idx = order[np.searchsorted(sorted_j, jj):
                        np.searchsorted(sorted_j, jj, side="right")]
            Xp[k * C:k * C + len(idx)] = X[idx]
            samp.append(idx)
        core_meta.append(samp)

        xT_in = np.ascontiguousarray(
            Xp.T.reshape(2, P, N).transpose(1, 0, 2).astype(NP_W))
        w1a_in = np.ascontiguousarray(
            np.stack([pack_w(W1a[jj]) for jj in judges], axis=1))
        w2a_in = np.ascontiguousarray(
            np.stack([pack_w(W2a[jj]) for jj in judges], axis=1))
        va_in = np.ascontiguousarray(
            np.stack([pack_w(Vaf[jj]) for jj in judges], axis=1))
        bm_in = np.empty((KB, NB + N), dtype=np.float32)
        bm_in[:JPC, :L1] = W1a[judges, 256]
        bm_in[:JPC, L1:L1 + L2] = W2a[judges, 256]
        bm_in[:JPC, L1 + L2:NB] = Vaf[judges, 256]
        bm_in[JPC, :NB] = np.concatenate([W1[256], W2[256], Vf[256]])
        bm_in[:, NB:] = mask_in
        in_maps.append({
            "xT": xT_in, "wsh": wsh_in,
            "w1a": w1a_in, "w2a": w2a_in, "va": va_in,
            "bm": bm_in.astype(NP_W),
        })

    res = run_bass_kernel_spmd(nc, in_maps, core_ids=list(range(N_CORES)))

    out = np.zeros((B, Q, A), dtype=np.float32)
    for c in range(N_CORES):
        oT = res.results[c]["outT"]          # [64, N]
        o = oT.T.reshape(N, Q, A)
        for k, idx in enumerate(core_meta[c]):
            out[idx] = o[k * C:k * C + len(idx)]
    return out
